# revision 11
# baseline (speedup 1.0000x reference)
"""Trainium2 Bass kernel for nn_ContinuousRNN.

Reference computation (B=256, N=2048, STEPS=64, DT=0.1):
    per step:
      phi = relu(x); g = relu(s); psi = relu(p)
      dx = -x + (g*phi) @ W_w.T + W_b
      ds = -s + w_proc_to_syn * psi + phi
      dp = -p + psi @ T_w.T + T_b + w_syn_to_proc * g
      x_n = x + DT*dx*free_inds ; s_n = p + DT*ds ; p_n = p + DT*dp
    returns (x_final, xs[STEPS,B,N])

Strategy: data-parallel over batch (32 rows/core on 8 cores), zero
cross-core communication. Per core, states live in a "folded batch-major"
layout [128, 512]: partition 32*t+b, col n  <->  (batch b, feature 512*t+n).
Matmuls are activation-stationary with 4-way PE column tiling
(tile_position=(0,32t)), streaming bf16 weights (SBUF-resident, 16MB)
with fp32 PSUM accumulation. Activations are transposed to feature-major
[128, 16, 32] via PE transpose for the stationary operand.
"""

import numpy as np
import ml_dtypes

import concourse.bass as bass
import concourse.mybir as mybir
from concourse import bacc
from concourse.tile import TileContext
from concourse.bass_utils import run_bass_kernel_spmd
from concourse.masks import make_identity

B, N, STEPS, DT = 256, 2048, 64, 0.1
NCORES = 8
BL = B // NCORES          # 32 batch rows per core
KB = N // 128             # 16 k-blocks
F32 = mybir.dt.float32
BF16 = mybir.dt.bfloat16
Relu = mybir.ActivationFunctionType.Relu
MULT = mybir.AluOpType.mult
ADD = mybir.AluOpType.add
SUB = mybir.AluOpType.subtract


def build_nc(steps=STEPS, col_tiled=True):
    # col_tiled=True: kb-outer/t-inner matmul order -> the 4 col-group tiles
    # stream concurrently on HW (tile_position concurrency), but their PSUM
    # accumulation groups interleave within one bank, which CoreSim's
    # partition-base-unaware pending-zero model mis-simulates -> only for HW.
    # col_tiled=False: t-outer/kb-inner, groups complete sequentially;
    # numerically identical, CoreSim-safe, slower on HW.
    nc = bacc.Bacc(None, target_bir_lowering=False)

    # per-core inputs (host pre-arranged layouts, see kernel() below)
    x0b = nc.declare_dram_parameter("x0b", [128, 512], F32, isOutput=False)
    dtfree = nc.declare_dram_parameter("dtfree", [128, 512], F32, isOutput=False)
    cxb = nc.declare_dram_parameter("cxb", [128, 512], F32, isOutput=False)
    c1t = nc.declare_dram_parameter("c1t", [128, 512], F32, isOutput=False)
    c2t = nc.declare_dram_parameter("c2t", [128, 512], F32, isOutput=False)
    c3t = nc.declare_dram_parameter("c3t", [128, 512], F32, isOutput=False)
    wwt = nc.declare_dram_parameter("wwt", [128, KB, 2048], BF16, isOutput=False)
    twt = nc.declare_dram_parameter("twt", [128, KB, 2048], BF16, isOutput=False)
    xs_out = nc.declare_dram_parameter("xs", [steps, 128, 512], F32, isOutput=True)

    with TileContext(nc) as tc:
        with (
            tc.tile_pool(name="wpool", bufs=1) as wpool,
            tc.tile_pool(name="spool", bufs=2) as spool,
            tc.tile_pool(name="tpool", bufs=2) as tpool,
            tc.tile_pool(name="apool", bufs=2) as apool,
            tc.tile_pool(name="pspool", bufs=2, space="PSUM") as pspool,
            tc.tile_pool(name="tppool", bufs=4, space="PSUM") as tppool,
        ):
            # resident weights + constants
            ww = wpool.tile([128, KB, 2048], BF16, name="ww")
            tw = wpool.tile([128, KB, 2048], BF16, name="tw")
            nc.sync.dma_start(ww[:], wwt[:])
            nc.sync.dma_start(tw[:], twt[:])
            cst = {}
            for nm, src in [("cxb", cxb), ("dtfree", dtfree), ("c1t", c1t),
                            ("c2t", c2t), ("c3t", c3t)]:
                t = wpool.tile([128, 512], F32, name=nm)
                nc.sync.dma_start(t[:], src[:])
                cst[nm] = t
            ident = wpool.tile([128, 128], BF16, name="ident")
            make_identity(nc, ident[:])

            x = spool.tile([128, 512], F32, name="x0", tag="x")
            s = spool.tile([128, 512], F32, name="s0", tag="s")
            p = spool.tile([128, 512], F32, name="p0", tag="p")
            nc.sync.dma_start(x[:], x0b[:])
            # init s/p via on-chip copies (not extra DMA queues): several
            # hardware instruction encodings (e.g. DVE scalar_tensor_tensor)
            # have a single embedded sync-wait slot, so step-0 consumers must
            # not need waits on two different producers.
            nc.vector.tensor_copy(s[:], x[:])
            nc.vector.tensor_copy(p[:], x[:])
            # Pre-observe init producers on the PE: after these, the first
            # real transpose/matmul each need at most one new wait.
            warmps = tppool.tile([128, 128], BF16, name="warmps", tag="tp")
            nc.tensor.transpose(warmps[:], ident[:], ident[:])
            warmps2 = tppool.tile([128, 128], F32, name="warmps2", tag="tp")
            nc.tensor.matmul(warmps2[:], ww[:, 0, :128], ident[:],
                             start=True, stop=True)
            warmps3 = tppool.tile([128, 128], F32, name="warmps3", tag="tp")
            nc.tensor.matmul(warmps3[:], tw[:, 0, :128], ident[:],
                             start=True, stop=True)

            def transpose_to_fmajor(act_b, act_f, nmpfx):
                # act_b: [128,512] bf16 batch-major; act_f: [128,KB,32] bf16
                # feature-major. 4 PE transposes of 128-col chunks + strided
                # copy-back regrouping col index q'=32t+b -> kb=4t+u.
                fview = act_f.rearrange("p (t u) b -> p u t b", u=4)
                for u in range(4):
                    tp = tppool.tile([128, 128], BF16, name=f"{nmpfx}{u}", tag="tp")
                    nc.tensor.transpose(tp[:], act_b[:, 128 * u:128 * (u + 1)],
                                        ident[:])
                    nc.scalar.copy(out=fview[:, u],
                                   in_=tp.rearrange("p (t b) -> p t b", t=4))

            def mm(act_f, w, psum, kk):
                # psum[32t+b, n] += sum_k act(b, k) * W[512t+n, k]
                order = ([(kb, t) for kb in range(KB) for t in range(4)]
                         if col_tiled else
                         [(kb, t) for t in range(4) for kb in range(KB)])
                for kb, t in order:
                    nc.tensor.matmul(
                        psum[32 * t:32 * (t + 1), :],
                        act_f[:, kb, :],
                        w[:, kb, 512 * t:512 * (t + 1)],
                        start=(kb == 0), stop=(kb == KB - 1),
                        tile_position=(0, 32 * t),
                        skip_group_check=col_tiled,
                    )
                del kk

            for st in range(steps):
                # --- activations (batch-major) ---
                phi = tpool.tile([128, 512], F32, name=f"phi{st}", tag="phi")
                g = tpool.tile([128, 512], F32, name=f"g{st}", tag="g")
                psi = tpool.tile([128, 512], F32, name=f"psi{st}", tag="psi")
                nc.scalar.activation(phi[:], x[:], Relu)
                nc.scalar.activation(g[:], s[:], Relu)
                a1b = apool.tile([128, 512], BF16, name=f"a1b{st}", tag="a1b")
                nc.vector.tensor_tensor(a1b[:], g[:], phi[:], MULT)
                a1f = apool.tile([128, KB, 32], BF16, name=f"a1f{st}", tag="a1f")
                transpose_to_fmajor(a1b, a1f, f"t1_{st}_")

                nc.scalar.activation(psi[:], p[:], Relu)
                a2b = apool.tile([128, 512], BF16, name=f"a2b{st}", tag="a2b")
                nc.scalar.activation(a2b[:], p[:], Relu)
                a2f = apool.tile([128, KB, 32], BF16, name=f"a2f{st}", tag="a2f")
                transpose_to_fmajor(a2b, a2f, f"t2_{st}_")

                # --- mm1 = (g*phi) @ W_w.T ---
                ps1 = pspool.tile([128, 512], F32, name=f"ps1_{st}", tag="ps1")
                mm(a1f, ww, ps1, 1)

                # --- x update: xn = x + dtfree*(ps1 + cxb - x); xs[st] = xn
                tA = tpool.tile([128, 512], F32, name=f"tA{st}", tag="tA")
                nc.vector.tensor_tensor(tA[:], ps1[:], x[:], SUB)
                nc.vector.tensor_tensor(tA[:], tA[:], cst["cxb"][:], ADD)
                nc.vector.tensor_tensor(tA[:], tA[:], cst["dtfree"][:], MULT)
                xn = spool.tile([128, 512], F32, name=f"x{st + 1}", tag="x")
                nc.vector.tensor_tensor(xn[:], tA[:], x[:], ADD)
                nc.sync.dma_start(xs_out[st], xn[:])

                # --- s update: sn = p + DT*(phi - s) + c1t*psi
                tB = tpool.tile([128, 512], F32, name=f"tB{st}", tag="tB")
                nc.vector.tensor_tensor(tB[:], phi[:], s[:], SUB)
                nc.scalar.mul(tB[:], tB[:], DT)
                nc.vector.tensor_tensor(tB[:], tB[:], p[:], ADD)
                tC = tpool.tile([128, 512], F32, name=f"tC{st}", tag="tC")
                nc.vector.tensor_tensor(tC[:], psi[:], cst["c1t"][:], MULT)
                sn = spool.tile([128, 512], F32, name=f"s{st + 1}", tag="s")
                nc.vector.tensor_tensor(sn[:], tB[:], tC[:], ADD)

                # --- mm2 = psi @ T_w.T ---
                ps2 = pspool.tile([128, 512], F32, name=f"ps2_{st}", tag="ps2")
                mm(a2f, tw, ps2, 2)

                # --- p update: pn = (1-DT)*p + c3t + DT*ps2 + c2t*g
                tD = tpool.tile([128, 512], F32, name=f"tD{st}", tag="tD")
                nc.scalar.mul(tD[:], p[:], 1.0 - DT)
                nc.vector.tensor_tensor(tD[:], tD[:], cst["c3t"][:], ADD)
                tE = tpool.tile([128, 512], F32, name=f"tE{st}", tag="tE")
                nc.scalar.mul(tE[:], ps2[:], DT)
                nc.vector.tensor_tensor(tD[:], tD[:], tE[:], ADD)
                nc.vector.tensor_tensor(tE[:], g[:], cst["c2t"][:], MULT)
                pn = spool.tile([128, 512], F32, name=f"p{st + 1}", tag="p")
                nc.vector.tensor_tensor(pn[:], tD[:], tE[:], ADD)

                x, s, p = xn, sn, pn

    nc.compile()
    return nc


def _fold_bm(a):
    # [32, 2048] -> folded batch-major [128, 512]: [32t+b, n] = a[b, 512t+n]
    return np.ascontiguousarray(
        a.reshape(BL, 4, 512).transpose(1, 0, 2).reshape(128, 512))


def _rep_feat(v):
    # [2048] per-feature vector -> folded batch-major [128, 512]
    return np.ascontiguousarray(
        np.broadcast_to(v.reshape(4, 1, 512), (4, BL, 512)).reshape(128, 512))


def _arrange_w(w):
    # W [j, k] -> [128, KB, 2048] bf16 with [p, kb, j] = W[j, 128*kb+p]
    return np.ascontiguousarray(
        w.T.reshape(KB, 128, 2048).transpose(1, 0, 2)).astype(ml_dtypes.bfloat16)


_cached = {}


def _get_nc(steps=STEPS):
    if steps not in _cached:
        _cached[steps] = build_nc(steps)
    return _cached[steps]


def make_in_maps(x0, free_inds, W_w, W_b, T_w, T_b, w_proc_to_syn, w_syn_to_proc):
    x0 = np.asarray(x0, np.float32)
    free_inds = np.asarray(free_inds, np.float32)
    shared = {
        "cxb": _rep_feat(np.asarray(W_b, np.float32)),
        "c1t": _rep_feat(DT * np.asarray(w_proc_to_syn, np.float32)),
        "c2t": _rep_feat(DT * np.asarray(w_syn_to_proc, np.float32)),
        "c3t": _rep_feat(DT * np.asarray(T_b, np.float32)),
        "wwt": _arrange_w(np.asarray(W_w, np.float32)),
        "twt": _arrange_w(np.asarray(T_w, np.float32)),
    }
    in_maps = []
    for c in range(NCORES):
        rows = slice(BL * c, BL * (c + 1))
        in_maps.append({
            "x0b": _fold_bm(x0[rows]),
            "dtfree": _fold_bm(DT * free_inds[rows]),
            **shared,
        })
    return in_maps


def unfold_xs(results, steps=STEPS):
    # per-core xs [steps,128,512] -> full xs [steps, B, N]
    xs = np.empty((steps, B, N), np.float32)
    for c, r in enumerate(results):
        a = r["xs"].reshape(steps, 4, BL, 512).transpose(0, 2, 1, 3)
        xs[:, BL * c:BL * (c + 1), :] = a.reshape(steps, BL, N)
    return xs


def kernel(x0, free_inds, W_w, W_b, T_w, T_b, w_proc_to_syn, w_syn_to_proc):
    nc = _get_nc(STEPS)
    in_maps = make_in_maps(x0, free_inds, W_w, W_b, T_w, T_b,
                           w_proc_to_syn, w_syn_to_proc)
    res = run_bass_kernel_spmd(nc, in_maps, list(range(NCORES)))
    xs = unfold_xs(res.results, STEPS)
    return xs[-1].copy(), xs


# revision 20
# speedup vs baseline: 6495.0631x; 6495.0631x over previous
"""Trainium2 Bass kernel for nn_ContinuousRNN.

Reference computation (B=256, N=2048, STEPS=64, DT=0.1):
    per step:
      phi = relu(x); g = relu(s); psi = relu(p)
      dx = -x + (g*phi) @ W_w.T + W_b
      ds = -s + w_proc_to_syn * psi + phi
      dp = -p + psi @ T_w.T + T_b + w_syn_to_proc * g
      x_n = x + DT*dx*free_inds ; s_n = p + DT*ds ; p_n = p + DT*dp
    returns (x_final, xs[STEPS,B,N])

Strategy: data-parallel over batch (32 rows/core on 8 cores), zero
cross-core communication. Per core, states live in a "folded batch-major"
layout [128, 512]: partition 32*t+b, col n  <->  (batch b, feature 512*t+n).
Matmuls are activation-stationary with 4-way PE column tiling
(tile_position=(0,32t)), streaming bf16 weights (SBUF-resident, 16MB)
with fp32 PSUM accumulation. Activations are transposed to feature-major
[128, 16, 32] via PE transpose for the stationary operand.
"""

import numpy as np
import ml_dtypes

import concourse.bass as bass
import concourse.mybir as mybir
from concourse import bacc
from concourse.tile import TileContext
from concourse.bass_utils import run_bass_kernel_spmd
from concourse.masks import make_identity

B, N, STEPS, DT = 256, 2048, 64, 0.1
NCORES = 8
BL = B // NCORES          # 32 batch rows per core
KB = N // 128             # 16 k-blocks
F32 = mybir.dt.float32
BF16 = mybir.dt.bfloat16
Relu = mybir.ActivationFunctionType.Relu
MULT = mybir.AluOpType.mult
ADD = mybir.AluOpType.add
SUB = mybir.AluOpType.subtract


def build_nc(steps=STEPS, col_tiled=True, repeats=1, ps_bufs=3, tp_bufs=2,
             sb_bufs=2):
    # col_tiled=True: kb-outer/t-inner matmul order -> the 4 col-group tiles
    # stream concurrently on HW (tile_position concurrency), but their PSUM
    # accumulation groups interleave within one bank, which CoreSim's
    # partition-base-unaware pending-zero model mis-simulates -> only for HW.
    # col_tiled=False: t-outer/kb-inner, groups complete sequentially;
    # numerically identical, CoreSim-safe, slower on HW.
    # repeats>1 wraps the recurrence in a For_i re-initializing state from x0
    # each iteration (xs overwritten) — timing-only, amortizes host overhead.
    nc = bacc.Bacc(None, target_bir_lowering=False)

    x0b = nc.declare_dram_parameter("x0b", [128, 512], F32, isOutput=False)
    dtfree = nc.declare_dram_parameter("dtfree", [128, 512], F32, isOutput=False)
    cxb = nc.declare_dram_parameter("cxb", [128, 512], F32, isOutput=False)
    c1t = nc.declare_dram_parameter("c1t", [128, 512], F32, isOutput=False)
    c2t = nc.declare_dram_parameter("c2t", [128, 512], F32, isOutput=False)
    c3t = nc.declare_dram_parameter("c3t", [128, 512], F32, isOutput=False)
    wwt = nc.declare_dram_parameter("wwt", [128, KB, 2048], BF16, isOutput=False)
    twt = nc.declare_dram_parameter("twt", [128, KB, 2048], BF16, isOutput=False)
    xs_out = nc.declare_dram_parameter("xs", [steps, 128, 512], F32, isOutput=True)

    with TileContext(nc) as tc:
        with (
            tc.tile_pool(name="wpool", bufs=1) as wpool,
            tc.tile_pool(name="spool", bufs=sb_bufs) as spool,
            tc.tile_pool(name="tpool", bufs=2) as tpool,
            tc.tile_pool(name="apool", bufs=sb_bufs) as apool,
            tc.tile_pool(name="pspool", bufs=ps_bufs, space="PSUM") as pspool,
            tc.tile_pool(name="tppool", bufs=tp_bufs, space="PSUM") as tppool,
        ):
            # resident weights + constants
            ww = wpool.tile([128, KB, 2048], BF16, name="ww")
            tw = wpool.tile([128, KB, 2048], BF16, name="tw")
            nc.sync.dma_start(ww[:], wwt[:])
            nc.sync.dma_start(tw[:], twt[:])
            cst = {}
            for nm, src in [("cxb", cxb), ("dtfree", dtfree), ("c1t", c1t),
                            ("c2t", c2t), ("c3t", c3t)]:
                t = wpool.tile([128, 512], F32, name=nm)
                nc.sync.dma_start(t[:], src[:])
                cst[nm] = t
            ident = wpool.tile([128, 128], BF16, name="ident")
            make_identity(nc, ident[:])

            # Pre-observe init producers on the PE so the first real
            # transpose/matmul each need at most one embedded sync wait.
            warmps = tppool.tile([128, 128], BF16, name="warmps", tag="tp")
            nc.tensor.transpose(warmps[:], ident[:], ident[:])
            warmps2 = tppool.tile([128, 128], F32, name="warmps2", tag="tp")
            nc.tensor.matmul(warmps2[:], ww[:, 0, :128], ident[:],
                             start=True, stop=True)
            warmps3 = tppool.tile([128, 128], F32, name="warmps3", tag="tp")
            nc.tensor.matmul(warmps3[:], tw[:, 0, :128], ident[:],
                             start=True, stop=True)

            def transpose_to_fmajor(act_b, act_f, nmpfx):
                # act_b: [128,512] bf16 batch-major; act_f: [128,KB,32] bf16
                # feature-major. 4 PE transposes of 128-col chunks + strided
                # copy-back regrouping col index q'=32t+b -> kb=4t+u.
                fview = act_f.rearrange("p (t u) b -> p u t b", u=4)
                for u in range(4):
                    tp = tppool.tile([128, 128], BF16, name=f"{nmpfx}{u}", tag="tp")
                    nc.tensor.transpose(tp[:], act_b[:, 128 * u:128 * (u + 1)],
                                        ident[:])
                    nc.scalar.copy(out=fview[:, u],
                                   in_=tp.rearrange("p (t b) -> p t b", t=4))

            def mm(act_f, w, psum):
                # psum[32t+b, n] += sum_k act(b, k) * W[512t+n, k]
                order = ([(kb, t) for kb in range(KB) for t in range(4)]
                         if col_tiled else
                         [(kb, t) for t in range(4) for kb in range(KB)])
                for kb, t in order:
                    nc.tensor.matmul(
                        psum[32 * t:32 * (t + 1), :],
                        act_f[:, kb, :],
                        w[:, kb, 512 * t:512 * (t + 1)],
                        start=(kb == 0), stop=(kb == KB - 1),
                        tile_position=(0, 32 * t),
                        skip_group_check=col_tiled,
                    )

            def step(st, x, s, p):
                # --- activations (batch-major) ---
                phi = tpool.tile([128, 512], F32, name=f"phi{st}", tag="phi")
                g = tpool.tile([128, 512], F32, name=f"g{st}", tag="g")
                psi = tpool.tile([128, 512], F32, name=f"psi{st}", tag="psi")
                nc.scalar.activation(phi[:], x[:], Relu)
                nc.scalar.activation(g[:], s[:], Relu)
                a1b = apool.tile([128, 512], BF16, name=f"a1b{st}", tag="a1b")
                nc.vector.tensor_tensor(a1b[:], g[:], phi[:], MULT)
                a1f = apool.tile([128, KB, 32], BF16, name=f"a1f{st}", tag="a1f")
                transpose_to_fmajor(a1b, a1f, f"t1_{st}_")

                nc.scalar.activation(psi[:], p[:], Relu)
                a2b = apool.tile([128, 512], BF16, name=f"a2b{st}", tag="a2b")
                nc.scalar.activation(a2b[:], p[:], Relu)
                a2f = apool.tile([128, KB, 32], BF16, name=f"a2f{st}", tag="a2f")
                transpose_to_fmajor(a2b, a2f, f"t2_{st}_")

                # --- mm1 = (g*phi) @ W_w.T ---
                ps1 = pspool.tile([128, 512], F32, name=f"ps1_{st}", tag="ps1")
                mm(a1f, ww, ps1)

                # --- x update: xn = x + dtfree*(ps1 + cxb - x); xs[st] = xn
                tA = tpool.tile([128, 512], F32, name=f"tA{st}", tag="tA")
                nc.vector.tensor_tensor(tA[:], ps1[:], x[:], SUB)
                nc.vector.tensor_tensor(tA[:], tA[:], cst["cxb"][:], ADD)
                nc.vector.tensor_tensor(tA[:], tA[:], cst["dtfree"][:], MULT)
                xn = spool.tile([128, 512], F32, name=f"x{st + 1}", tag="x")
                nc.vector.tensor_tensor(xn[:], tA[:], x[:], ADD)
                nc.sync.dma_start(xs_out[st], xn[:])

                # --- s update: sn = p + DT*(phi - s) + c1t*psi
                tB = tpool.tile([128, 512], F32, name=f"tB{st}", tag="tB")
                nc.vector.tensor_tensor(tB[:], phi[:], s[:], SUB)
                nc.scalar.mul(tB[:], tB[:], DT)
                nc.vector.tensor_tensor(tB[:], tB[:], p[:], ADD)
                tC = tpool.tile([128, 512], F32, name=f"tC{st}", tag="tC")
                nc.vector.tensor_tensor(tC[:], psi[:], cst["c1t"][:], MULT)
                sn = spool.tile([128, 512], F32, name=f"s{st + 1}", tag="s")
                nc.vector.tensor_tensor(sn[:], tB[:], tC[:], ADD)

                # --- mm2 = psi @ T_w.T ---
                ps2 = pspool.tile([128, 512], F32, name=f"ps2_{st}", tag="ps2")
                mm(a2f, tw, ps2)

                # --- p update: pn = (1-DT)*p + c3t + DT*ps2 + c2t*g
                tD = tpool.tile([128, 512], F32, name=f"tD{st}", tag="tD")
                nc.scalar.mul(tD[:], p[:], 1.0 - DT)
                nc.vector.tensor_tensor(tD[:], tD[:], cst["c3t"][:], ADD)
                tE = tpool.tile([128, 512], F32, name=f"tE{st}", tag="tE")
                nc.scalar.mul(tE[:], ps2[:], DT)
                nc.vector.tensor_tensor(tD[:], tD[:], tE[:], ADD)
                nc.vector.tensor_tensor(tE[:], g[:], cst["c2t"][:], MULT)
                pn = spool.tile([128, 512], F32, name=f"p{st + 1}", tag="p")
                nc.vector.tensor_tensor(pn[:], tD[:], tE[:], ADD)
                return xn, sn, pn

            def recurrence():
                x = spool.tile([128, 512], F32, name="xin", tag="x")
                s = spool.tile([128, 512], F32, name="sin", tag="s")
                p = spool.tile([128, 512], F32, name="pin", tag="p")
                nc.sync.dma_start(x[:], x0b[:])
                # s/p via on-chip copies (not extra DMA queues): hardware
                # instruction encodings have limited embedded sync-wait
                # slots, so step-0 consumers must not need waits on two
                # different DMA queues.
                nc.vector.tensor_copy(s[:], x[:])
                nc.vector.tensor_copy(p[:], x[:])
                for st in range(steps):
                    x, s, p = step(st, x, s, p)

            if repeats == 1:
                recurrence()
            else:
                with tc.For_i(0, repeats, 1):
                    recurrence()

    nc.compile()
    return nc


def _fold_bm(a):
    # [32, 2048] -> folded batch-major [128, 512]: [32t+b, n] = a[b, 512t+n]
    return np.ascontiguousarray(
        a.reshape(BL, 4, 512).transpose(1, 0, 2).reshape(128, 512))


def _rep_feat(v):
    # [2048] per-feature vector -> folded batch-major [128, 512]
    return np.ascontiguousarray(
        np.broadcast_to(v.reshape(4, 1, 512), (4, BL, 512)).reshape(128, 512))


def _arrange_w(w):
    # W [j, k] -> [128, KB, 2048] bf16 with [p, kb, j] = W[j, 128*kb+p]
    return np.ascontiguousarray(
        w.T.reshape(KB, 128, 2048).transpose(1, 0, 2)).astype(ml_dtypes.bfloat16)


_cached = {}


def _get_nc(steps=STEPS):
    if steps not in _cached:
        _cached[steps] = build_nc(steps)
    return _cached[steps]


def make_in_maps(x0, free_inds, W_w, W_b, T_w, T_b, w_proc_to_syn, w_syn_to_proc):
    x0 = np.asarray(x0, np.float32)
    free_inds = np.asarray(free_inds, np.float32)
    shared = {
        "cxb": _rep_feat(np.asarray(W_b, np.float32)),
        "c1t": _rep_feat(DT * np.asarray(w_proc_to_syn, np.float32)),
        "c2t": _rep_feat(DT * np.asarray(w_syn_to_proc, np.float32)),
        "c3t": _rep_feat(DT * np.asarray(T_b, np.float32)),
        "wwt": _arrange_w(np.asarray(W_w, np.float32)),
        "twt": _arrange_w(np.asarray(T_w, np.float32)),
    }
    in_maps = []
    for c in range(NCORES):
        rows = slice(BL * c, BL * (c + 1))
        in_maps.append({
            "x0b": _fold_bm(x0[rows]),
            "dtfree": _fold_bm(DT * free_inds[rows]),
            **shared,
        })
    return in_maps


def unfold_xs(results, steps=STEPS):
    # per-core xs [steps,128,512] -> full xs [steps, B, N]
    xs = np.empty((steps, B, N), np.float32)
    for c, r in enumerate(results):
        a = r["xs"].reshape(steps, 4, BL, 512).transpose(0, 2, 1, 3)
        xs[:, BL * c:BL * (c + 1), :] = a.reshape(steps, BL, N)
    return xs


def kernel(x0, free_inds, W_w, W_b, T_w, T_b, w_proc_to_syn, w_syn_to_proc):
    nc = _get_nc(STEPS)
    in_maps = make_in_maps(x0, free_inds, W_w, W_b, T_w, T_b,
                           w_proc_to_syn, w_syn_to_proc)
    res = run_bass_kernel_spmd(nc, in_maps, list(range(NCORES)))
    xs = unfold_xs(res.results, STEPS)
    return xs[-1].copy(), xs
